# revision 1
# baseline (speedup 1.0000x reference)
"""Trainium2 Bass kernel for the FD (facilitation-depression) synapse layer.

Reference computes, per (b, h) lane, a sequential recurrence over T timesteps
with K=4 unrolled Euler substeps:

    Ca_diff = Ca - Ca_mu
    sig     = sigmoid(Ca_diff / Ca_sigma)
    temp    = P_rel_max*sig * R * I_t
    EPSC    = EPSC - dt*(EPSC/tau_EPSC + beta*temp)
    R       = R + dt*((k_min + k_delta*sig)*(1-R) - temp)
    Ca      = Ca + dt*(alpha*I_t - Ca_diff/tau_Ca)

Key structure exploited here:
  * Each (b,h) lane is independent -> 16384 parallel lanes.
  * Ca is linear with constant per-lane coefficient and independent of R/EPSC:
    Ca_{s+1} = c1*Ca_s + u_t  (substeps s, u_t = dt*alpha*I_t + dt/tau_Ca*mu).
  * Given sig, R is linear time-varying:  R_{k+1} = P_k*R_k + Q_k  with
    P_k = (1-dt*k_min) - sig_k*(dt*k_delta + dt*Prm*I_t),  Q_k = dt*k_min +
    dt*k_delta*sig_k.
  * Given R, EPSC is linear with constant coefficient e1 = 1-dt/tau_EPSC and
    input -dt*beta*Prm*I_t*sig_k*R_k.  The within-timestep accumulation is
    done with a Horner scheme (weights e1^{3-k}) so the EPSC scan runs at
    timestep granularity and its output is directly the kernel output.
  * All three recurrences use the DVE's native tensor_tensor_scan (fp32
    internal state) with time along the free dimension.
  * The timestep->substep broadcast of V is materialized by an ACT Identity
    op with a step-0 (replicating) input AP, so vsig runs as a dense bf16
    tensor_tensor.
  * Engine balance (GPSIMD legally runs only plain tensor_tensor/copy/
    memset): DVE does the scans + scalar_tensor_tensor (comb, Horner) +
    tensor_scalar ops; ACT does sigmoid, Q and the I_t affine precomputes;
    GPSIMD does the two big dense multiplies (vsig, sr), with ~6 instances
    steered back to the DVE to equalize engine busy time.
  * The per-(lb, blk) work is emitted as a 4-stage software pipeline
    (A at unit i, B1 at i-3, BH at i-5, B2 at i-7) so cross-engine
    latency (ACT sigmoid, GPSIMD products) never stalls the DVE.

Sharding: batch 32 -> 4 samples per core (pure data parallel). Per core the
4*512 = 2048 lanes are processed as 16 lane-batches of 128 partitions; time is
blocked at TB timesteps with scan-state carried across blocks.

Host side does all parameter transforms and the (b,t,h) <-> (lane, t)
transposes in numpy; device time is pure compute + contiguous DMA.
"""

import numpy as np
from contextlib import ExitStack

import concourse.bass as bass
import concourse.mybir as mybir
import concourse.tile as tile
from concourse.bass_utils import run_bass_kernel_spmd

f32 = mybir.dt.float32
bf16 = mybir.dt.bfloat16
AF = mybir.ActivationFunctionType
OP = mybir.AluOpType

B, T, H = 32, 2048, 512
K = 4               # ode substeps per timestep
NCORES = 8
BPC = B // NCORES   # batch per core (4)
GH = H // 128       # h-groups of 128 (4)
NLB = BPC * GH      # lane batches per core (16)
PD = 128            # partitions
TB = 512            # timesteps per block
NPAR = 20

# param column indices
(C1, G1, G2, G3, SC0, SC1, SC2, SC3, BIAS, UC,
 UA, SV, AV, CP, QM, QA, SW2, E1, E14, CA0) = range(NPAR)


def build_program(Tn=T, tb=TB, nlb=NLB, n_devices=NCORES):
    """Build the Bass program (SPMD; same program on every core)."""
    nblk = Tn // tb
    S = K * tb
    nc = bass.Bass("TRN2", target_bir_lowering=False, debug=False,
                   num_devices=n_devices)
    I_d = nc.dram_tensor("i_ca", [nlb, PD, Tn], f32, kind="ExternalInput").ap()
    par_d = nc.dram_tensor("par", [PD, nlb * NPAR], f32,
                           kind="ExternalInput").ap()
    # one output tensor per (lb, blk) keeps every out-DMA to a single RAW dep
    O_d = [[nc.dram_tensor(f"epsc_{lb}_{blk}", [PD, tb], f32,
                           kind="ExternalOutput").ap()
            for blk in range(nblk)] for lb in range(nlb)]

    with ExitStack() as ctx:
        tc = ctx.enter_context(tile.TileContext(nc))
        apool = ctx.enter_context(tc.tile_pool(name="ahand", bufs=6))
        mpool = ctx.enter_context(tc.tile_pool(name="amid", bufs=4))
        bpool = ctx.enter_context(tc.tile_pool(name="bshort", bufs=4))
        cpool = ctx.enter_context(tc.tile_pool(name="bcarry", bufs=4))
        ipool = ctx.enter_context(tc.tile_pool(name="inp", bufs=2))
        ppool = ctx.enter_context(tc.tile_pool(name="par", bufs=1))

        par = ppool.tile([PD, nlb * NPAR], f32, tag="par")
        nc.sync.dma_start(par[:], par_d)

        itile_lbs = {}
        prev_cap = {}          # per-lb Ca scan carry (stage A chain)
        prev_rb = {}           # per-lb (rsh, etile) carries (stage B chain)

        def pcol_of(lb):
            return lambda i: par[:, lb * NPAR + i:lb * NPAR + i + 1]

        def stage_a(lb, blk):
            """I-precomputes, Ca scan, comb, sigmoid, Q — no dependence on
            stage B. Returns the tiles stage B needs."""
            pcol = pcol_of(lb)
            t0 = blk * tb
            if blk == 0:
                itile_lb = ipool.tile([PD, Tn], f32, tag="itile")
                nc.sync.dma_start(itile_lb[:], I_d[lb])
                itile_lbs[lb] = itile_lb
            itile = itile_lbs[lb][:, t0:t0 + tb]

            u = mpool.tile([PD, tb], f32, tag="u")
            nc.scalar.activation(u[:], itile, AF.Identity,
                                 bias=pcol(UA), scale=pcol(UC))
            V = apool.tile([PD, tb], bf16, tag="V")
            nc.scalar.activation(V[:], itile, AF.Identity,
                                 bias=pcol(AV), scale=pcol(SV))
            W2 = apool.tile([PD, tb], bf16, tag="W2")
            nc.scalar.activation(W2[:], itile, AF.Copy, scale=pcol(SW2))

            # Ca' scan (timestep granularity, Ca' = Ca/S4);
            # capsh[:, 0] = carry-in; capsh[:, 0:tb] = Ca'_t at timestep start
            capsh = mpool.tile([PD, tb + 1], f32, tag="capsh")
            if blk == 0:
                nc.vector.tensor_copy(capsh[:, 0:1], pcol(CA0))
            else:
                nc.vector.tensor_copy(capsh[:, 0:1],
                                      prev_cap[lb][:, tb:tb + 1])
            nc.vector.tensor_tensor_scan(
                capsh[:, 1:tb + 1], pcol(C1).to_broadcast((PD, tb)), u[:],
                capsh[:, 0:1], OP.mult, OP.add)
            prev_cap[lb] = capsh
            cap0 = capsh[:, 0:tb]

            # substep sigmoid inputs: comb_k = Ca'_t * g_k + u_t  (k=1..3)
            comb = mpool.tile([PD, 3, tb], bf16, tag="comb")
            for k in (1, 2, 3):
                nc.vector.scalar_tensor_tensor(
                    comb[:, k - 1], cap0, pcol((G1, G2, G3)[k - 1]), u[:],
                    OP.mult, OP.add)

            # sig[s], s = 4t+k (k inner); per-k ACT calls absorb c1^k
            sig = apool.tile([PD, S], bf16, tag="sig")
            sig3 = sig[:].rearrange("p (t k) -> p t k", k=K)
            nc.scalar.activation(sig3[:, :, 0], cap0, AF.Sigmoid,
                                 bias=pcol(BIAS), scale=pcol(SC0))
            for k in (1, 2, 3):
                nc.scalar.activation(sig3[:, :, k], comb[:, k - 1],
                                     AF.Sigmoid, bias=pcol(BIAS),
                                     scale=pcol((SC1, SC2, SC3)[k - 1]))

            vsig = bpool.tile([PD, S], bf16, tag="vsig")
            nc.gpsimd.tensor_mul(
                vsig[:].rearrange("p (t k) -> p t k", k=K),
                sig[:].rearrange("p (t k) -> p t k", k=K),
                V[:].unsqueeze(2).broadcast_to((PD, tb, K)))
            return sig, vsig, W2

        def stage_b1(lb, blk, sig, vsig, W2):
            """P, Q, R scan, sr (on Pool)."""
            pcol = pcol_of(lb)

            Pt = bpool.tile([PD, S], bf16, tag="Pt")
            nc.vector.tensor_scalar(Pt[:], vsig[:], -1.0, pcol(CP),
                                    OP.mult, OP.add)
            Qt = bpool.tile([PD, S], bf16, tag="Qt")
            nc.scalar.activation(Qt[:], sig[:], AF.Identity,
                                 bias=pcol(QA), scale=pcol(QM))

            # R scan (substep granularity); rsh[:,0] = carry-in
            rsh = cpool.tile([PD, S + 1], bf16, tag="rsh")
            if blk == 0:
                nc.vector.memset(rsh[:, 0:1], 1.0)
            else:
                nc.vector.tensor_copy(rsh[:, 0:1],
                                      prev_rb[lb][:, S:S + 1])
            nc.vector.tensor_tensor_scan(rsh[:, 1:S + 1], Pt[:], Qt[:],
                                         rsh[:, 0:1], OP.mult, OP.add)
            prev_rb[lb] = rsh

            # sr_s = sig_s * R_s (R at substep start = shifted scan out)
            sr = bpool.tile([PD, S], bf16, tag="sr")
            seng = nc.vector if (blk + nblk * lb) % 8 == 3 else nc.gpsimd
            seng.tensor_mul(sr[:], sig[:], rsh[:, 0:S])
            return sr, W2

        def stage_bh(lb, blk, sr, W2):
            """Horner over k + racc (DVE), one unit after sr (Pool)."""
            pcol = pcol_of(lb)
            srk = sr[:].rearrange("p (t k) -> p t k", k=K)
            h1 = bpool.tile([PD, tb], bf16, tag="h1")
            nc.vector.scalar_tensor_tensor(h1[:], srk[:, :, 0], pcol(E1),
                                           srk[:, :, 1], OP.mult, OP.add)
            h2 = bpool.tile([PD, tb], bf16, tag="h2")
            nc.vector.scalar_tensor_tensor(h2[:], h1[:], pcol(E1),
                                           srk[:, :, 2], OP.mult, OP.add)
            sacc = bpool.tile([PD, tb], bf16, tag="sacc")
            nc.vector.scalar_tensor_tensor(sacc[:], h2[:], pcol(E1),
                                           srk[:, :, 3], OP.mult, OP.add)
            racc = cpool.tile([PD, tb], bf16, tag="racc")
            nc.vector.tensor_mul(racc[:], W2[:], sacc[:])
            return racc

        def stage_b2(lb, blk, racc):
            """EPSC scan + out-DMA (after the GPSIMD chain drains)."""
            pcol = pcol_of(lb)
            etile = cpool.tile([PD, tb], f32, tag="etile")
            einit = 0.0 if blk == 0 else prev_e[lb][:, tb - 1:tb]
            nc.vector.tensor_tensor_scan(
                etile[:], pcol(E14).to_broadcast((PD, tb)), racc[:],
                einit, OP.mult, OP.add)
            prev_e[lb] = etile
            nc.sync.dma_start(O_d[lb][blk][:], etile[:])

        prev_e = {}
        # two-deep software skew: A(i+2) and B1(i+1) are emitted before
        # B2(i), so the DVE never stalls on the ACT sigmoid (A->B1) or the
        # GPSIMD product chain (B1->B2)
        units = [(lb, blk) for lb in range(nlb) for blk in range(nblk)]
        n = len(units)
        a_out = {}
        b1_out = {}
        bh_out = {}
        for i in range(n + 7):
            if i < n:
                a_out[i] = stage_a(*units[i])
            if 3 <= i <= n + 2:
                b1_out[i - 3] = stage_b1(*units[i - 3], *a_out.pop(i - 3))
            if 5 <= i <= n + 4:
                bh_out[i - 5] = stage_bh(*units[i - 5], *b1_out.pop(i - 5))
            if i >= 7:
                stage_b2(*units[i - 7], bh_out.pop(i - 7))

    # Split multi-wait sync conditions into InstEventSemaphore carriers --
    # TRN2 instructions encode at most 1 sync wait (2 for EventSemaphore).
    import bass_rust
    bass_rust.generate_event_semaphores(nc)
    return nc


def derive_params(log_Ca_mu, log_Ca_sigma, log_tau_Ca, log_alpha, log_tau_EPSC,
                  log_beta, presigmoid_P_rel_max, log_k_recov_min,
                  log_k_recov_delta, ode_steps):
    """Host-side param math (fp64), returns [H, NPAR] fp32."""
    d = np.float64
    dt = 1.0 / int(ode_steps)
    mu = np.exp(log_Ca_mu.astype(d))
    sigma = np.exp(log_Ca_sigma.astype(d))
    tau_Ca = np.exp(log_tau_Ca.astype(d))
    alpha = np.exp(log_alpha.astype(d))
    tau_E = np.exp(log_tau_EPSC.astype(d))
    beta = np.exp(log_beta.astype(d))
    Prm = 1.0 / (1.0 + np.exp(-presigmoid_P_rel_max.astype(d)))
    k_min = np.exp(log_k_recov_min.astype(d))
    k_delta = np.exp(log_k_recov_delta.astype(d))

    c1 = 1.0 - dt / tau_Ca
    S1 = np.ones_like(c1)
    S2 = 1.0 + c1
    S3 = 1.0 + c1 + c1 ** 2
    S4 = S3 + c1 ** 3
    e1 = 1.0 - dt / tau_E

    n = log_Ca_mu.shape[0]
    par = np.zeros((n, NPAR), np.float64)
    par[:, C1] = c1 ** 4                 # Ca' scan coefficient (timesteps)
    par[:, G1] = c1 * S4 / S1
    par[:, G2] = c1 ** 2 * S4 / S2
    par[:, G3] = c1 ** 3 * S4 / S3
    par[:, SC0] = S4 / sigma
    par[:, SC1] = S1 / sigma
    par[:, SC2] = S2 / sigma
    par[:, SC3] = S3 / sigma
    par[:, BIAS] = -mu / sigma
    par[:, UC] = dt * alpha
    par[:, UA] = dt / tau_Ca * mu
    par[:, SV] = dt * Prm
    par[:, AV] = dt * k_delta
    par[:, CP] = 1.0 - dt * k_min
    par[:, QM] = dt * k_delta
    par[:, QA] = dt * k_min
    par[:, SW2] = -dt * beta * Prm
    par[:, E1] = e1
    par[:, E14] = e1 ** 4
    par[:, CA0] = mu / S4                # Ca'_0
    return par.astype(np.float32)


_PROG = None
LAST_RESULTS = None  # BassKernelResults of the most recent kernel() call


def _get_program():
    global _PROG
    if _PROG is None:
        _PROG = build_program()
    return _PROG


def kernel(I_Ca, log_Ca_mu, log_Ca_sigma, log_tau_Ca, log_alpha, log_tau_EPSC,
           log_beta, presigmoid_P_rel_max, log_k_recov_min, log_k_recov_delta,
           ode_steps):
    assert int(ode_steps) == K, f"kernel hardcodes {K} substeps"
    I_Ca = np.asarray(I_Ca, np.float32)
    assert I_Ca.shape == (B, T, H)

    par_h = derive_params(
        np.asarray(log_Ca_mu), np.asarray(log_Ca_sigma), np.asarray(log_tau_Ca),
        np.asarray(log_alpha), np.asarray(log_tau_EPSC), np.asarray(log_beta),
        np.asarray(presigmoid_P_rel_max), np.asarray(log_k_recov_min),
        np.asarray(log_k_recov_delta), ode_steps)          # [H, NPAR]

    # lane-batch lb = b_local*GH + g holds lanes h = g*128 + p
    par_lb = par_h.reshape(GH, PD, NPAR)                    # [g, p, NPAR]
    par_core = np.ascontiguousarray(
        np.broadcast_to(par_lb[None], (BPC, GH, PD, NPAR)).reshape(
            NLB, PD, NPAR).transpose(1, 0, 2).reshape(PD, NLB * NPAR))

    nc = _get_program()
    in_maps = []
    for c in range(NCORES):
        Ic = I_Ca[c * BPC:(c + 1) * BPC]                    # [BPC, T, H]
        # -> [b_local, g, p, t]
        Ic = Ic.reshape(BPC, T, GH, PD).transpose(0, 2, 3, 1)
        in_maps.append({
            "i_ca": np.ascontiguousarray(Ic.reshape(NLB, PD, T)),
            "par": par_core,
        })

    res = run_bass_kernel_spmd(nc, in_maps, core_ids=list(range(NCORES)))
    global LAST_RESULTS
    LAST_RESULTS = res
    nblk = T // TB
    out = np.empty((B, T, H), np.float32)
    for c in range(NCORES):
        Oc = np.stack([
            np.concatenate([res.results[c][f"epsc_{lb}_{blk}"]
                            for blk in range(nblk)], axis=1)
            for lb in range(NLB)])                          # [NLB, PD, T]
        Oc = Oc.reshape(BPC, GH, PD, T)
        out[c * BPC:(c + 1) * BPC] = Oc.transpose(0, 3, 1, 2).reshape(BPC, T, H)
    return out

